# revision 46
# baseline (speedup 1.0000x reference)
"""ArcFace combined-margin loss kernel for 8 TRN2 NeuronCores.

Strategy
--------
reference: cos = (f @ w.T) / (|f||w|); phi = arcface(cos);
outputs = s*(labels*phi + (1-labels)*cos); loss = mean over rows of
-(sum of log_softmax(outputs) at lab_pinds, masked) / L^2.

labels is the multi-hot of (lab_pinds, lengths), so outputs differs from
s*cos only at <=8 entries/row.  The only device-scale compute is the
dense denominator  sexp[b] = sum_c exp(30*cos[b,c] - 30)  (B*C*D MACs +
B*C exps).  Everything else is O(B*L*D + C*D) and runs on host float64.

Device (per core, C-sharded: 2500 classes/core zero-padded to 2560):
  inputs are HOST-prepared fp8 operands, pre-normalized, pre-scaled and
  pre-transposed so the exp argument has a CONSTANT scale/bias:
     fT8[d, b] = fp8(30 * f[b,d] / |f_b|)      [512, 2048]
     wT8[d, c] = fp8(16 * w[c,d] / |w_c|)      [512, 2560]  (class shard)
  dot_psum = sum_d fT8*wT8 = 480*cos, so exp arg = dot/16 - 30 for every
  element -- ACT instructions need no per-row scale and can span any
  PSUM bank group.  Loop: 80 (row-block m, class-chunk n) tiles in
  block-major order; each tile = 2 fp8 DoubleRow matmuls (K=256) into
  one PSUM bank; groups of 4 tiles (4 banks, double-buffered 4+4) are
  evicted by one ACT Exp -> bf16 SBUF strip; DVE tensor_reduce sums each
  block's 2560-wide strip into sexp[128, 16].
Host (numpy float64): positive dots f.w[pinds] exactly, arcface margin,
denominator correction at positives, logsumexp, masked ragged CE, mean.
No collectives (the only cross-core reduction is summing 8 sexp
partials on host during unsharding).
"""

import math
import sys

import numpy as np
import ml_dtypes

for _p in ("/opt/trn_rl_repo",):
    if _p not in sys.path:
        sys.path.append(_p)

import concourse.bass as bass
import concourse.bacc as bacc
import concourse.mybir as mybir
import concourse.tile as tile
from concourse.bass_utils import run_bass_kernel_spmd
from contextlib import ExitStack

B, C, D, LMAX = 2048, 20000, 512, 8
NCORES = 8
CSH = C // NCORES          # 2500 real classes per core
CSHP = 2560                # padded to 5*512 (bank-aligned chunks)
NBLK = B // 128            # 16 row blocks
NW = 512                   # matmul N-chunk width (exactly one PSUM bank)
NCH = CSHP // NW           # 5 chunks per core
KC = D // 128              # 4 contraction chunks (128 partitions each)
NT = NBLK * NCH            # 80 (m, n) tiles
NG = NT // 4               # 20 ACT eviction groups of 4 banks
NRED = 10                  # blocks whose row-sum is reduced on-device (DVE);
                           # the last NBLK-NRED blocks ship raw bf16 strips
                           # to the host (hides the DVE reduce tail)
S = 30.0
M_MARGIN = 0.5
FSC = 30.0                 # f rows scaled to 30*unit
WSC = 16.0                 # w rows scaled to 16*unit
# psum dot = FSC*WSC*cos; exp arg = dot/WSC - 30 = 30*cos - 30

F32 = mybir.dt.float32
BF16 = mybir.dt.bfloat16
FP8 = mybir.dt.float8e4
E4M3 = ml_dtypes.float8_e4m3

_GRAPH = None


def build_graph():
    nc = bacc.Bacc()
    # host supplies operands already in SBUF layout: [p, k, col] with
    # element (p, k, c) = x[c, k*128+p], so one DMA covers all k-chunks
    fT_ext = nc.declare_dram_parameter("fT8", [128, KC, B], FP8, isOutput=False)
    wT_ext = nc.declare_dram_parameter("wT8", [128, KC, CSHP], FP8, isOutput=False)
    sexp_ext = nc.declare_dram_parameter("sexp", [128, NRED], F32, isOutput=True)
    strips_ext = nc.declare_dram_parameter(
        "strips", [NBLK - NRED, 128, NCH, NW], BF16, isOutput=True
    )

    AF = mybir.ActivationFunctionType

    with ExitStack() as ctx:
        tc = ctx.enter_context(tile.TileContext(nc))
        const = ctx.enter_context(tc.tile_pool(name="const", bufs=1))
        resident = ctx.enter_context(tc.tile_pool(name="resident", bufs=1))
        pmm = ctx.enter_context(tc.tile_pool(name="pmm", bufs=2, space="PSUM"))
        scr = ctx.enter_context(tc.tile_pool(name="scr", bufs=2))

        nbias = const.tile([128, 1], F32)
        nc.vector.memset(nbias[:], -S)
        dact = const.tile([128, 1], BF16)

        fT = resident.tile([128, KC, B], FP8)
        wT = resident.tile([128, KC, CSHP], FP8)
        strip = resident.tile([128, NT, NW], BF16)
        sexp_t = resident.tile([128, NRED], F32)

        # DMA pieces ordered by when the compute needs them (tile order is
        # block-major, so tile 0-1 need fT col-group 0 + wT chunks 0-1 for
        # all four k-chunks first).  Issues round-robin over the 3 DMA-capable
        # engine queues (each dma_start costs ~600ns serially per queue).
        # wave-1 (fT cols 0-511 + wT chunk 0, needed by the first quad) goes
        # as 8 small per-k pieces for DGE parallelism; the rest as 5 big
        # column-range DMAs.  Scalar queue only early (free before 1st Exp).
        qs = [nc.sync, nc.gpsimd, nc.scalar]
        for k in range(KC):
            qs[(2 * k) % 3].dma_start(
                fT[:, k, 0:NW], fT_ext[:, k, 0:NW]
            )
            qs[(2 * k + 1) % 3].dma_start(
                wT[:, k, 0:NW], wT_ext[:, k, 0:NW]
            )
        pieces = [
            (2, wT, wT_ext, NW, NW),          # wT chunk 1
            (0, wT, wT_ext, 2 * NW, NW),      # wT chunk 2
            (1, wT, wT_ext, 3 * NW, NW),      # wT chunk 3
            (2, wT, wT_ext, 4 * NW, NW),      # wT chunk 4
            (0, fT, fT_ext, NW, NW),          # fT cols 512-1023
            (1, fT, fT_ext, 2 * NW, 2 * NW),  # fT cols rest
        ]
        for q, t, ext, c0, cw in pieces:
            qs[q].dma_start(t[:, :, c0 : c0 + cw], ext[:, :, c0 : c0 + cw])
        # preload the Exp activation table off the critical path
        nc.scalar.activation(dact[:], nbias[:], AF.Exp, bias=nbias[:], scale=1.0)

        # warm up the PE while the input DMAs land: the tensor engine's
        # clock p-state ramps with sustained use (0.65 -> 1.2 -> 2.4 GHz);
        # without this the PE paces the whole pipeline at ~1.2 GHz and ACT
        # waits ~400ns per eviction group.
        warm = const.tile([128, 2, NW], FP8)
        nc.vector.memset(warm[:], 0.0)
        Pw = pmm.tile([128, 4, NW], F32, tag="mm", name="warm")
        for i in range(6):
            nc.tensor.matmul(
                Pw[:, i % 4, :],
                warm[:, :, 0:128],
                warm[:],
                start=True,
                stop=True,
                perf_mode=mybir.MatmulPerfMode.DoubleRow,
            )

        # main loop.  Execution order: the first 10 tiles sweep column pairs
        # (m0,n),(m1,n) for n = 0..4 so each wT chunk is consumed right as
        # its DMA lands; the rest is block-major over m = 2..15.  Strip slot
        # for tile (m, n) is always 5m+n, so block reduces read contiguous
        # slots (pair-group ACT writes are stride-5).  Super-periods are
        # k2-outer (PE stationary constant across same-block matmuls) and
        # split into <=4-bank subgroups, one ACT Exp each.
        etiles = [(h, n) for n in range(NCH) for h in range(4)] + [
            (4 + q // NCH, q % NCH) for q in range(NT - 4 * NCH)
        ]
        sps = [(0, 2), (2, 2)] + [(4 * i, 4) for i in range(1, 5)] + [
            (20 + 8 * i, 8) for i in range(7)
        ] + [(76, 4)]
        nout = 0
        for s, (t0, sz) in enumerate(sps):
            subs = [
                (t0 + off, min(4, sz - off)) for off in range(0, sz, 4)
            ]  # (subgroup start, width)
            Ps = [
                pmm.tile([128, w, NW], F32, tag="mm", name=f"mm{g0}")
                for g0, w in subs
            ]
            for k2 in range(KC // 2):
                for j in range(sz):
                    m, n = etiles[t0 + j]
                    nc.tensor.matmul(
                        Ps[j // 4][:, j % 4, :],
                        fT[:, 2 * k2 : 2 * k2 + 2, m * 128 : (m + 1) * 128],
                        wT[:, 2 * k2 : 2 * k2 + 2, n * NW : (n + 1) * NW],
                        start=(k2 == 0),
                        stop=(k2 == KC // 2 - 1),
                        perf_mode=mybir.MatmulPerfMode.DoubleRow,
                    )
            for (g0, w), P in zip(subs, Ps):
                slots = [NCH * m + n for m, n in etiles[g0 : g0 + w]]
                st = slots[1] - slots[0] if w > 1 else 1
                nc.scalar.activation(
                    strip[:, slots[0] : slots[-1] + 1 : st, :], P[:], AF.Exp,
                    bias=nbias[:], scale=1.0 / WSC,
                )
                # late blocks' strips stream to the host as soon as each
                # eviction group lands (keeps the final transfer tiny)
                for m in range(NRED, NBLK):
                    lo = max(slots[0], NCH * m) - NCH * m
                    hi = min(slots[-1] + 1, NCH * m + NCH) - NCH * m
                    if st == 1 and lo < hi:
                        # last super-periods' pieces issue from the scalar
                        # queue (free right after its own last Exp); the rest
                        # from gpsimd, whose issues track the ACT cadence
                        q = nc.scalar if s == len(sps) - 1 else nc.gpsimd
                        nout += 1
                        q.dma_start(
                            strips_ext[m - NRED][:, lo:hi, :],
                            strip[:, NCH * m + lo : NCH * m + hi, :],
                        )
            for m in range(NBLK):
                done = 16 + m if m < 4 else NCH * m + NCH - 1
                if t0 <= done < t0 + sz:
                    if m < NRED:
                        # row-sum on DVE (1x, ~2.8us/block, hidden under ACT)
                        sc = scr.tile([128, NCH, NW], BF16, tag="scr")
                        nc.vector.tensor_scalar(
                            sc[:],
                            strip[:, NCH * m : NCH * m + NCH, :],
                            1.0,
                            0.0,
                            op0=mybir.AluOpType.mult,
                            op1=mybir.AluOpType.add,
                            accum_out=sexp_t[:, m : m + 1],
                        )
                        if m == NRED - 1:
                            nc.sync.dma_start(sexp_ext[:, :], sexp_t[:, :NRED])

    nc.finalize()
    return nc


def _get_graph():
    global _GRAPH
    if _GRAPH is None:
        _GRAPH = build_graph()
    return _GRAPH


def make_in_maps(f, lab_word2vec, lab_pinds=None):
    f = np.asarray(f, dtype=np.float32)
    w = np.asarray(lab_word2vec, dtype=np.float32)
    fn = np.sqrt((f.astype(np.float64) ** 2).sum(axis=1))
    wn = np.sqrt((w.astype(np.float64) ** 2).sum(axis=1))
    # [p, k, col] SBUF layout: element (p, k, c) = x[c, k*128+p]
    fT8 = np.ascontiguousarray(
        (f * (FSC / fn)[:, None].astype(np.float32)).T.astype(E4M3)
        .reshape(KC, 128, B).transpose(1, 0, 2)
    )
    w8 = (w * (WSC / wn)[:, None].astype(np.float32)).astype(E4M3)
    in_maps = []
    for i in range(NCORES):
        wT8 = np.zeros((128, KC, CSHP), dtype=E4M3)
        wT8[:, :, :CSH] = (
            w8[i * CSH : (i + 1) * CSH].T.reshape(KC, 128, CSH).transpose(1, 0, 2)
        )
        in_maps.append({"fT8": fT8, "wT8": wT8})
    return in_maps


def combine(outs, f, lab_word2vec, lab_pinds, lengths):
    """outs: list of 8 dicts with sexp [128, NBLK]. Returns float32 loss."""
    f = np.asarray(f, dtype=np.float64)
    w = np.asarray(lab_word2vec, dtype=np.float64)
    pinds = np.asarray(lab_pinds, dtype=np.int64)
    lens = np.asarray(lengths, dtype=np.int64)

    # s_shift[b] = sum_c exp(30 cos - 30); b = m*128 + p
    s_shift = np.zeros(B, dtype=np.float64)
    for i in range(NCORES):
        per_block = np.empty((128, NBLK), dtype=np.float64)
        per_block[:, :NRED] = outs[i]["sexp"].astype(np.float64)
        # late blocks shipped as raw bf16 exp strips; sum on host
        per_block[:, NRED:] = (
            outs[i]["strips"].astype(np.float64).sum(axis=(2, 3)).T
        )
        s_shift += per_block.T.reshape(B)
    # the 60 zero-pad classes per core contribute exp(-30) each (cos = 0)
    s_shift -= NCORES * (CSHP - CSH) * math.exp(-S)

    fn = np.sqrt((f * f).sum(axis=1))     # [B]
    wn = np.sqrt((w * w).sum(axis=1))     # [C]
    pd = np.einsum("bjd,bd->bj", w[pinds], f)              # [B, LMAX]
    cos = pd / np.maximum(fn[:, None] * wn[pinds], 1e-8)

    cos_m, sin_m = math.cos(M_MARGIN), math.sin(M_MARGIN)
    th = math.cos(math.pi - M_MARGIN)
    mm = math.sin(math.pi - M_MARGIN) * M_MARGIN
    sine = np.sqrt(np.clip(1.0 - cos * cos, 0.0, 1.0))
    phi = cos * cos_m - sine * sin_m
    phi = np.where(cos > th, phi, cos - mm)

    mask = (np.arange(LMAX)[None, :] < lens[:, None]).astype(np.float64)
    corr = (mask * (np.exp(S * phi - S) - np.exp(S * cos - S))).sum(axis=1)
    z = S + np.log(s_shift + corr)  # logsumexp of outputs, [B]
    pos_sum = (mask * (S * phi)).sum(axis=1)
    L = lens.astype(np.float64)
    per_sample = (L * z - pos_sum) / (L * L)
    return np.float32(per_sample.mean())


def kernel(f, labels, lab_word2vec, lab_pinds, lengths):
    nc = _get_graph()
    in_maps = make_in_maps(f, lab_word2vec)
    res = run_bass_kernel_spmd(nc, in_maps, core_ids=list(range(NCORES)))
    return combine(res.results, f, lab_word2vec, lab_pinds, lengths)


# revision 47
# speedup vs baseline: 1.1828x; 1.1828x over previous
"""ArcFace combined-margin loss kernel for 8 TRN2 NeuronCores.

Strategy
--------
reference: cos = (f @ w.T) / (|f||w|); phi = arcface(cos);
outputs = s*(labels*phi + (1-labels)*cos); loss = mean over rows of
-(sum of log_softmax(outputs) at lab_pinds, masked) / L^2.

labels is the multi-hot of (lab_pinds, lengths), so outputs differs from
s*cos only at <=8 entries/row.  The only device-scale compute is the
dense denominator  sexp[b] = sum_c exp(30*cos[b,c] - 30)  (B*C*D MACs +
B*C exps).  Everything else is O(B*L*D + C*D) and runs on host float64.

Device (per core, C-sharded: 2500 classes/core zero-padded to 2560):
  inputs are HOST-prepared fp8 operands, pre-normalized, pre-scaled and
  pre-transposed so the exp argument has a CONSTANT scale/bias:
     fT8[d, b] = fp8(30 * f[b,d] / |f_b|)      [512, 2048]
     wT8[d, c] = fp8(16 * w[c,d] / |w_c|)      [512, 2560]  (class shard)
  dot_psum = sum_d fT8*wT8 = 480*cos, so exp arg = dot/16 - 30 for every
  element -- ACT instructions need no per-row scale and can span any
  PSUM bank group.  Loop: 80 (row-block m, class-chunk n) tiles in
  block-major order; each tile = 2 fp8 DoubleRow matmuls (K=256) into
  one PSUM bank; groups of 4 tiles (4 banks, double-buffered 4+4) are
  evicted by one ACT Exp -> bf16 SBUF strip; DVE tensor_reduce sums each
  block's 2560-wide strip into sexp[128, 16].
Host (numpy float64): positive dots f.w[pinds] exactly, arcface margin,
denominator correction at positives, logsumexp, masked ragged CE, mean.
No collectives (the only cross-core reduction is summing 8 sexp
partials on host during unsharding).
"""

import math
import sys

import numpy as np
import ml_dtypes

for _p in ("/opt/trn_rl_repo",):
    if _p not in sys.path:
        sys.path.append(_p)

import concourse.bass as bass
import concourse.bacc as bacc
import concourse.mybir as mybir
import concourse.tile as tile
from concourse.bass_utils import run_bass_kernel_spmd
from contextlib import ExitStack

B, C, D, LMAX = 2048, 20000, 512, 8
NCORES = 8
CSH = C // NCORES          # 2500 real classes per core
CSHP = 2560                # padded to 5*512 (bank-aligned chunks)
NBLK = B // 128            # 16 row blocks
NW = 512                   # matmul N-chunk width (exactly one PSUM bank)
NCH = CSHP // NW           # 5 chunks per core
KC = D // 128              # 4 contraction chunks (128 partitions each)
NT = NBLK * NCH            # 80 (m, n) tiles
NG = NT // 4               # 20 ACT eviction groups of 4 banks
NRED = 10                  # blocks whose row-sum is reduced on-device (DVE);
                           # the last NBLK-NRED blocks ship raw bf16 strips
                           # to the host (hides the DVE reduce tail)
S = 30.0
M_MARGIN = 0.5
FSC = 30.0                 # f rows scaled to 30*unit
WSC = 16.0                 # w rows scaled to 16*unit
# psum dot = FSC*WSC*cos; exp arg = dot/WSC - 30 = 30*cos - 30

F32 = mybir.dt.float32
BF16 = mybir.dt.bfloat16
FP8 = mybir.dt.float8e4
E4M3 = ml_dtypes.float8_e4m3

_GRAPH = None


def build_graph():
    nc = bacc.Bacc()
    # host supplies operands already in SBUF layout: [p, k, col] with
    # element (p, k, c) = x[c, k*128+p], so one DMA covers all k-chunks
    fT_ext = nc.declare_dram_parameter("fT8", [128, KC, B], FP8, isOutput=False)
    wT_ext = nc.declare_dram_parameter("wT8", [128, KC, CSHP], FP8, isOutput=False)
    sexp_ext = nc.declare_dram_parameter("sexp", [128, NRED], F32, isOutput=True)
    strips_ext = nc.declare_dram_parameter(
        "strips", [NBLK - NRED, 128, NCH, NW], BF16, isOutput=True
    )

    AF = mybir.ActivationFunctionType

    with ExitStack() as ctx:
        tc = ctx.enter_context(tile.TileContext(nc))
        const = ctx.enter_context(tc.tile_pool(name="const", bufs=1))
        resident = ctx.enter_context(tc.tile_pool(name="resident", bufs=1))
        pmm = ctx.enter_context(tc.tile_pool(name="pmm", bufs=2, space="PSUM"))
        scr = ctx.enter_context(tc.tile_pool(name="scr", bufs=2))

        nbias = const.tile([128, 1], F32)
        nc.vector.memset(nbias[:], -S)
        dact = const.tile([128, 1], BF16)

        fT = resident.tile([128, KC, B], FP8)
        wT = resident.tile([128, KC, CSHP], FP8)
        strip = resident.tile([128, NT, NW], BF16)
        sexp_t = resident.tile([128, NRED], F32)

        # DMA pieces ordered by when the compute needs them (tile order is
        # block-major, so tile 0-1 need fT col-group 0 + wT chunks 0-1 for
        # all four k-chunks first).  Issues round-robin over the 3 DMA-capable
        # engine queues (each dma_start costs ~600ns serially per queue).
        # wave-1 (fT cols 0-511 + wT chunk 0, needed by the first quad) goes
        # as 8 small per-k pieces for DGE parallelism; the rest as 5 big
        # column-range DMAs.  Scalar queue only early (free before 1st Exp).
        qs = [nc.sync, nc.gpsimd, nc.scalar]
        for k in range(KC):
            qs[(2 * k) % 3].dma_start(
                fT[:, k, 0:NW], fT_ext[:, k, 0:NW]
            )
            qs[(2 * k + 1) % 3].dma_start(
                wT[:, k, 0:NW], wT_ext[:, k, 0:NW]
            )
        pieces = [
            (2, wT, wT_ext, NW, NW),          # wT chunk 1
            (0, wT, wT_ext, 2 * NW, NW),      # wT chunk 2
            (1, wT, wT_ext, 3 * NW, NW),      # wT chunk 3
            (2, wT, wT_ext, 4 * NW, NW),      # wT chunk 4
            (0, fT, fT_ext, NW, NW),          # fT cols 512-1023
            (1, fT, fT_ext, 2 * NW, 2 * NW),  # fT cols rest
        ]
        for q, t, ext, c0, cw in pieces:
            qs[q].dma_start(t[:, :, c0 : c0 + cw], ext[:, :, c0 : c0 + cw])
        # preload the Exp activation table off the critical path
        nc.scalar.activation(dact[:], nbias[:], AF.Exp, bias=nbias[:], scale=1.0)

        # warm up the PE while the input DMAs land: the tensor engine's
        # clock p-state ramps with sustained use (0.65 -> 1.2 -> 2.4 GHz);
        # without this the PE paces the whole pipeline at ~1.2 GHz and ACT
        # waits ~400ns per eviction group.
        warm = const.tile([128, 2, NW], FP8)
        nc.vector.memset(warm[:], 0.0)
        Pw = pmm.tile([128, 4, NW], F32, tag="mm", name="warm")
        for i in range(6):
            nc.tensor.matmul(
                Pw[:, i % 4, :],
                warm[:, :, 0:128],
                warm[:],
                start=True,
                stop=True,
                perf_mode=mybir.MatmulPerfMode.DoubleRow,
            )

        # main loop.  Execution order: the first 10 tiles sweep column pairs
        # (m0,n),(m1,n) for n = 0..4 so each wT chunk is consumed right as
        # its DMA lands; the rest is block-major over m = 2..15.  Strip slot
        # for tile (m, n) is always 5m+n, so block reduces read contiguous
        # slots (pair-group ACT writes are stride-5).  Super-periods are
        # k2-outer (PE stationary constant across same-block matmuls) and
        # split into <=4-bank subgroups, one ACT Exp each.
        etiles = [(h, n) for n in range(NCH) for h in range(4)] + [
            (4 + q // NCH, q % NCH) for q in range(NT - 4 * NCH)
        ]
        sps = [(4 * i, 4) for i in range(5)] + [
            (20 + 8 * i, 8) for i in range(7)
        ] + [(76, 4)]
        nout = 0
        for s, (t0, sz) in enumerate(sps):
            subs = [
                (t0 + off, min(4, sz - off)) for off in range(0, sz, 4)
            ]  # (subgroup start, width)
            Ps = [
                pmm.tile([128, w, NW], F32, tag="mm", name=f"mm{g0}")
                for g0, w in subs
            ]
            for k2 in range(KC // 2):
                for j in range(sz):
                    m, n = etiles[t0 + j]
                    nc.tensor.matmul(
                        Ps[j // 4][:, j % 4, :],
                        fT[:, 2 * k2 : 2 * k2 + 2, m * 128 : (m + 1) * 128],
                        wT[:, 2 * k2 : 2 * k2 + 2, n * NW : (n + 1) * NW],
                        start=(k2 == 0),
                        stop=(k2 == KC // 2 - 1),
                        perf_mode=mybir.MatmulPerfMode.DoubleRow,
                    )
            for (g0, w), P in zip(subs, Ps):
                slots = [NCH * m + n for m, n in etiles[g0 : g0 + w]]
                st = slots[1] - slots[0] if w > 1 else 1
                nc.scalar.activation(
                    strip[:, slots[0] : slots[-1] + 1 : st, :], P[:], AF.Exp,
                    bias=nbias[:], scale=1.0 / WSC,
                )
                # late blocks' strips stream to the host as soon as each
                # eviction group lands (keeps the final transfer tiny)
                for m in range(NRED, NBLK):
                    lo = max(slots[0], NCH * m) - NCH * m
                    hi = min(slots[-1] + 1, NCH * m + NCH) - NCH * m
                    if st == 1 and lo < hi:
                        # last super-periods' pieces issue from the scalar
                        # queue (free right after its own last Exp); the rest
                        # from gpsimd, whose issues track the ACT cadence
                        q = nc.scalar if s == len(sps) - 1 else nc.gpsimd
                        nout += 1
                        q.dma_start(
                            strips_ext[m - NRED][:, lo:hi, :],
                            strip[:, NCH * m + lo : NCH * m + hi, :],
                        )
            for m in range(NBLK):
                done = 16 + m if m < 4 else NCH * m + NCH - 1
                if t0 <= done < t0 + sz:
                    if m < NRED:
                        # row-sum on DVE (1x, ~2.8us/block, hidden under ACT)
                        sc = scr.tile([128, NCH, NW], BF16, tag="scr")
                        nc.vector.tensor_scalar(
                            sc[:],
                            strip[:, NCH * m : NCH * m + NCH, :],
                            1.0,
                            0.0,
                            op0=mybir.AluOpType.mult,
                            op1=mybir.AluOpType.add,
                            accum_out=sexp_t[:, m : m + 1],
                        )
                        if m == NRED - 1:
                            nc.sync.dma_start(sexp_ext[:, :], sexp_t[:, :NRED])

    nc.finalize()
    return nc


def _get_graph():
    global _GRAPH
    if _GRAPH is None:
        _GRAPH = build_graph()
    return _GRAPH


def make_in_maps(f, lab_word2vec, lab_pinds=None):
    f = np.asarray(f, dtype=np.float32)
    w = np.asarray(lab_word2vec, dtype=np.float32)
    fn = np.sqrt((f.astype(np.float64) ** 2).sum(axis=1))
    wn = np.sqrt((w.astype(np.float64) ** 2).sum(axis=1))
    # [p, k, col] SBUF layout: element (p, k, c) = x[c, k*128+p]
    fT8 = np.ascontiguousarray(
        (f * (FSC / fn)[:, None].astype(np.float32)).T.astype(E4M3)
        .reshape(KC, 128, B).transpose(1, 0, 2)
    )
    w8 = (w * (WSC / wn)[:, None].astype(np.float32)).astype(E4M3)
    in_maps = []
    for i in range(NCORES):
        wT8 = np.zeros((128, KC, CSHP), dtype=E4M3)
        wT8[:, :, :CSH] = (
            w8[i * CSH : (i + 1) * CSH].T.reshape(KC, 128, CSH).transpose(1, 0, 2)
        )
        in_maps.append({"fT8": fT8, "wT8": wT8})
    return in_maps


def combine(outs, f, lab_word2vec, lab_pinds, lengths):
    """outs: list of 8 dicts with sexp [128, NBLK]. Returns float32 loss."""
    f = np.asarray(f, dtype=np.float64)
    w = np.asarray(lab_word2vec, dtype=np.float64)
    pinds = np.asarray(lab_pinds, dtype=np.int64)
    lens = np.asarray(lengths, dtype=np.int64)

    # s_shift[b] = sum_c exp(30 cos - 30); b = m*128 + p
    s_shift = np.zeros(B, dtype=np.float64)
    for i in range(NCORES):
        per_block = np.empty((128, NBLK), dtype=np.float64)
        per_block[:, :NRED] = outs[i]["sexp"].astype(np.float64)
        # late blocks shipped as raw bf16 exp strips; sum on host
        per_block[:, NRED:] = (
            outs[i]["strips"].astype(np.float64).sum(axis=(2, 3)).T
        )
        s_shift += per_block.T.reshape(B)
    # the 60 zero-pad classes per core contribute exp(-30) each (cos = 0)
    s_shift -= NCORES * (CSHP - CSH) * math.exp(-S)

    fn = np.sqrt((f * f).sum(axis=1))     # [B]
    wn = np.sqrt((w * w).sum(axis=1))     # [C]
    pd = np.einsum("bjd,bd->bj", w[pinds], f)              # [B, LMAX]
    cos = pd / np.maximum(fn[:, None] * wn[pinds], 1e-8)

    cos_m, sin_m = math.cos(M_MARGIN), math.sin(M_MARGIN)
    th = math.cos(math.pi - M_MARGIN)
    mm = math.sin(math.pi - M_MARGIN) * M_MARGIN
    sine = np.sqrt(np.clip(1.0 - cos * cos, 0.0, 1.0))
    phi = cos * cos_m - sine * sin_m
    phi = np.where(cos > th, phi, cos - mm)

    mask = (np.arange(LMAX)[None, :] < lens[:, None]).astype(np.float64)
    corr = (mask * (np.exp(S * phi - S) - np.exp(S * cos - S))).sum(axis=1)
    z = S + np.log(s_shift + corr)  # logsumexp of outputs, [B]
    pos_sum = (mask * (S * phi)).sum(axis=1)
    L = lens.astype(np.float64)
    per_sample = (L * z - pos_sum) / (L * L)
    return np.float32(per_sample.mean())


def kernel(f, labels, lab_word2vec, lab_pinds, lengths):
    nc = _get_graph()
    in_maps = make_in_maps(f, lab_word2vec)
    res = run_bass_kernel_spmd(nc, in_maps, core_ids=list(range(NCORES)))
    return combine(res.results, f, lab_word2vec, lab_pinds, lengths)
